# revision 13
# baseline (speedup 1.0000x reference)
"""Trainium2 Bass kernel for one dense transformer block (B=4, S=2048, D=768, H=12).

Sharding: 8 cores = 4 batches x 2 interleaved sequence halves, no collectives.
Core (b, h) owns 8 of the 16 128-token blocks of batch b, interleaved
(h=0: [0,3,4,7,8,11,12,15], h=1: [1,2,5,6,9,10,13,14]) so causal attention
work is balanced. Each core recomputes LN1 + K/V over the full 2048-token
context and runs attention + MLP for its own 1024 queries.

Everything on-device is feature-major ([features-on-partitions, tokens-free]),
which is exactly the operand layout the tensor engine needs, so the kernel has
no on-device transposes. The host pre-transposes x, folds LN gains/biases and
the V bias into adjacent weights, and builds the causal masks for the two
boundary key tiles of every query block. Softmax denominators come from a ones
column appended to V; softmax needs no max-subtraction because the scores are
scaled by 1/sqrt(768) and tiny. rstd = exp(-0.5*ln(var+eps)) keeps LayerNorm
off the slow iterative-divide path.
"""

import math

import numpy as np
import ml_dtypes

import concourse.bass as bass
import concourse.mybir as mybir
import concourse.tile as tile
from concourse.bass_utils import run_bass_kernel_spmd
from concourse.vector_clock import ScopedClock

AF = mybir.ActivationFunctionType
ALU = mybir.AluOpType
BF16 = mybir.dt.bfloat16
F32 = mybir.dt.float32

B, S, D, H = 4, 2048, 768, 12
HD = D // H          # 64
EPS = 1e-5
P = 128
KSUB = D // P        # 6
NB = S // P          # 16 query blocks per batch
SQ = S // 2          # 1024 own queries per core
N_CORES = 8
SCALE = 1.0 / math.sqrt(D)


def own_blocks(h: int) -> list[int]:
    out = []
    for j in range(4):
        out += [4 * j, 4 * j + 3] if h == 0 else [4 * j + 1, 4 * j + 2]
    return out


class SplitDrainTileContext(tile.TileContext):
    """walrus here rejects a Drain carrying >1 sync-wait; split the kernel-tail
    drain into one Drain per semaphore wait."""

    def _drain_and_barrier(self, tick_clock, wait_clock):
        nc = self.nc
        drain_inst = nc.sync.drain()
        wait_clock.add_sem_waits(
            drain_inst.ins, ScopedClock({None: tick_clock.global_clock})
        )
        nc.all_engine_barrier()
        assert self.sems is not None
        popped = nc._tile_sem_poison_stack.pop()
        assert popped is self._sem_poison
        nc.clear_and_free_semaphores(list(self.sems.allocated().values()))
        nc.all_engine_barrier()
        self._split_multi_waits(nc)

    @staticmethod
    def _split_multi_waits(nc):
        """walrus here accepts at most one sync-wait command per instruction;
        hoist extra waits onto same-engine NoOps spliced just before."""
        k = 0
        for bb in nc.main_func.blocks:
            out = []
            for ins in bb.instructions:
                si = ins.sync_info
                if si is not None and si.on_wait and len(si.on_wait) > 1:
                    waits = list(si.on_wait)
                    si.on_wait = [waits[-1]]
                    for w in waits[:-1]:
                        k += 1
                        out.append(
                            mybir.InstNoOp(
                                name=f"{ins.name}_sw{k}",
                                engine=ins.engine,
                                ins=[],
                                outs=[],
                                sync_info=mybir.SyncInfo(on_wait=[w], on_update=[]),
                            )
                        )
                out.append(ins)
            try:
                bb.instructions[:] = out
            except TypeError:
                bb.set_instructions(out)


def _ln_stats_apply(nc, lnp, lnrows, psum_stat, psum_bc, xch, xnch, ones, ones_row, eps_row):
    """One 512-token LayerNorm chunk, feature-major.

    xch: SBUF [128, KSUB, 512] f32 source; xnch: SBUF [128, KSUB, 512] bf16
    out = (x - mu(x)) * rstd(x). Sums over features (partitions) via
    ones-matmuls; rstd = exp(-0.5*ln(var+eps)).
    """
    ps_s = psum_stat.tile([1, 512], F32, tag="ln_ps")
    ps_q = psum_stat.tile([1, 512], F32, tag="ln_ps")
    for j in range(KSUB):
        nc.tensor.matmul(
            ps_s[:], ones[:], xch[:, j, :], start=(j == 0), stop=(j == KSUB - 1)
        )
    for j in range(KSUB):
        sq = lnp.tile([P, 512], F32, tag="ln_sq")
        nc.scalar.activation(sq[:], xch[:, j, :], AF.Square)
        nc.tensor.matmul(
            ps_q[:], ones[:], sq[:], start=(j == 0), stop=(j == KSUB - 1)
        )
    mu = lnrows.tile([1, 512], F32, tag="ln_mu")
    nc.vector.tensor_scalar_mul(mu[:], ps_s[:], 1.0 / D)
    m2 = lnrows.tile([1, 512], F32, tag="ln_m2")
    nc.vector.tensor_mul(m2[:], mu[:], mu[:])
    var = lnrows.tile([1, 512], F32, tag="ln_var")
    nc.vector.scalar_tensor_tensor(
        out=var[:], in0=ps_q[:], scalar=1.0 / D, in1=m2[:],
        op0=ALU.mult, op1=ALU.subtract,
    )
    lnv = lnrows.tile([1, 512], F32, tag="ln_lnv")
    nc.scalar.activation(lnv[:], var[:], AF.Ln, bias=eps_row[:])
    rstd = lnrows.tile([1, 512], F32, tag="ln_rstd")
    nc.scalar.activation(rstd[:], lnv[:], AF.Exp, scale=-0.5)
    negmur = lnrows.tile([1, 512], F32, tag="ln_negmur")
    nc.vector.scalar_tensor_tensor(
        out=negmur[:], in0=mu[:], scalar=-1.0, in1=rstd[:],
        op0=ALU.mult, op1=ALU.mult,
    )
    rstd_b = psum_bc.tile([P, 512], F32, tag="bc")
    nc.tensor.matmul(rstd_b[:], ones_row[:], rstd[:], start=True, stop=True)
    negmur_b = psum_bc.tile([P, 512], F32, tag="bc")
    nc.tensor.matmul(negmur_b[:], ones_row[:], negmur[:], start=True, stop=True)
    nc.vector.tensor_tensor(
        xnch[:], xch[:], rstd_b[:, None, :].to_broadcast([P, KSUB, 512]), ALU.mult
    )
    nc.vector.tensor_tensor(
        xnch[:], xnch[:], negmur_b[:, None, :].to_broadcast([P, KSUB, 512]), ALU.add
    )


def build_nc():
    nc = bass.Bass("TRN2", num_devices=N_CORES)
    xT = nc.declare_dram_parameter("xT", [P, KSUB, S], F32, isOutput=False)
    xTq = nc.declare_dram_parameter("xTq", [P, KSUB, SQ], F32, isOutput=False)
    wqkv = nc.declare_dram_parameter("wqkv", [P, KSUB, 3 * D], BF16, isOutput=False)
    wod = nc.declare_dram_parameter("wo", [P, KSUB, D], BF16, isOutput=False)
    w1d = nc.declare_dram_parameter("w1", [P, KSUB, 4 * D], BF16, isOutput=False)
    w2d = nc.declare_dram_parameter("w2", [P, 4 * KSUB, D], BF16, isOutput=False)
    bqkd = nc.declare_dram_parameter("bqk", [P, 12], F32, isOutput=False)
    bod = nc.declare_dram_parameter("bo", [P, KSUB], F32, isOutput=False)
    b1d = nc.declare_dram_parameter("b1", [P, 4 * KSUB], F32, isOutput=False)
    b2d = nc.declare_dram_parameter("b2", [P, KSUB], F32, isOutput=False)
    maskd = nc.declare_dram_parameter("mask", [NB, P, P], BF16, isOutput=False)
    sel2d = nc.declare_dram_parameter("sel2", [2, P], F32, isOutput=False)
    yT = nc.declare_dram_parameter("yT", [P, KSUB, SQ], F32, isOutput=True)

    with SplitDrainTileContext(nc) as tc:
        # LEFT stack: long-lived small pools; h1 and pt released mid-kernel
        persist = tc.alloc_tile_pool(name="persist", bufs=1, side="left")
        lnp = tc.alloc_tile_pool(name="lnp", bufs=2, side="left")
        lnx = tc.alloc_tile_pool(name="lnx", bufs=1, side="left")
        lnrows = tc.alloc_tile_pool(name="lnrows", bufs=1, side="left")
        xnp = tc.alloc_tile_pool(name="xnp", bufs=2, side="left")
        stage = tc.alloc_tile_pool(name="stage", bufs=2, side="left")
        h1pool = tc.alloc_tile_pool(name="h1", bufs=1, side="left")
        ptpool = tc.alloc_tile_pool(name="pt", bufs=3, side="left")
        # RIGHT stack: phase-scoped big pools
        pool_attn = tc.alloc_tile_pool(name="attn", bufs=1, side="right")
        pool_w = tc.alloc_tile_pool(name="wqkv", bufs=1, side="right")
        psum_mm = tc.alloc_tile_pool(name="psum_mm", bufs=2, space="PSUM")
        psum_ctx = tc.alloc_tile_pool(name="psum_ctx", bufs=2, space="PSUM")
        psum_stat = tc.alloc_tile_pool(name="psum_stat", bufs=2, space="PSUM")
        psum_bc = tc.alloc_tile_pool(name="psum_bc", bufs=2, space="PSUM")

        ones = persist.tile([P, 1], F32)
        nc.vector.memset(ones[:], 1.0)
        eps_row = persist.tile([1, 1], F32)
        nc.vector.memset(eps_row[:], EPS)
        ones_row = persist.tile([1, P], F32)
        nc.vector.memset(ones_row[:], 1.0)
        # selection matrix: rb[p] = row0 for p<64, row1 for p>=64
        sel2 = persist.tile([2, P], F32)
        nc.sync.dma_start(out=sel2[:], in_=sel2d[:])
        bqk_s = persist.tile([P, 12], F32)
        nc.sync.dma_start(out=bqk_s[:], in_=bqkd[:])
        bo_s = persist.tile([P, KSUB], F32)
        nc.sync.dma_start(out=bo_s[:], in_=bod[:])
        b1_s = persist.tile([P, 4 * KSUB], F32)
        nc.sync.dma_start(out=b1_s[:], in_=b1d[:])
        b2_s = persist.tile([P, KSUB], F32)
        nc.sync.dma_start(out=b2_s[:], in_=b2d[:])

        wqkv_s = pool_w.tile([P, KSUB, 3 * D], BF16)
        nc.sync.dma_start(out=wqkv_s[:], in_=wqkv[:])
        mask_s = pool_attn.tile([P, NB, P], BF16)
        nc.sync.dma_start(out=mask_s[:], in_=maskd[:].rearrange("t k q -> k t q"))

        kT = pool_attn.tile([P, KSUB, S], BF16)
        vaug = pool_attn.tile([P, NB, H, HD + 1], BF16)
        qT = pool_attn.tile([P, KSUB, SQ], BF16)

        # ======== LN1(ctx chunk) fused with K-proj and V-proj ========
        for c0 in range(0, S, 512):
            xch = lnx.tile([P, KSUB, 512], F32, tag="ln_x")
            nc.sync.dma_start(out=xch[:], in_=xT[:, :, c0 : c0 + 512])
            xnch = xnp.tile([P, KSUB, 512], BF16, tag="xnch")
            _ln_stats_apply(nc, lnp, lnrows, psum_stat, psum_bc, xch, xnch, ones, ones_row, eps_row)
            # K-proj: kT[:, m, c0:c0+512]
            for m in range(KSUB):
                ps = psum_mm.tile([P, 512], F32, tag="mm")
                for j in range(KSUB):
                    nc.tensor.matmul(
                        ps[:],
                        wqkv_s[:, j, D + 128 * m : D + 128 * (m + 1)],
                        xnch[:, j, :],
                        start=(j == 0),
                        stop=(j == KSUB - 1),
                    )
                nc.scalar.activation(
                    kT[:, m, c0 : c0 + 512], ps[:], AF.Identity,
                    bias=bqk_s[:, 6 + m : 7 + m],
                )
            # V-proj: 128-token subchunks, natural layout, per-head stride 65
            for t in range(4):
                tt = c0 // P + t
                for half in range(2):
                    ps = psum_mm.tile([P, 384], F32, tag="mm")
                    for j in range(KSUB):
                        nc.tensor.matmul(
                            ps[:],
                            xnch[:, j, 128 * t : 128 * (t + 1)],
                            wqkv_s[:, j, 2 * D + 384 * half : 2 * D + 384 * (half + 1)],
                            start=(j == 0),
                            stop=(j == KSUB - 1),
                        )
                    nc.vector.tensor_copy(
                        vaug[:, tt, 6 * half : 6 * (half + 1), 0:HD],
                        ps[:].rearrange("p (h d) -> p h d", d=HD),
                    )
        nc.vector.memset(vaug[:, :, :, HD : HD + 1], 1.0)

        # ======== LN1(own chunk) fused with Q-proj ========
        for c0 in range(0, SQ, 512):
            xch = lnx.tile([P, KSUB, 512], F32, tag="ln_x")
            nc.sync.dma_start(out=xch[:], in_=xTq[:, :, c0 : c0 + 512])
            xnch = xnp.tile([P, KSUB, 512], BF16, tag="xnch")
            _ln_stats_apply(nc, lnp, lnrows, psum_stat, psum_bc, xch, xnch, ones, ones_row, eps_row)
            for m in range(KSUB):
                ps = psum_mm.tile([P, 512], F32, tag="mm")
                for j in range(KSUB):
                    nc.tensor.matmul(
                        ps[:],
                        wqkv_s[:, j, 128 * m : 128 * (m + 1)],
                        xnch[:, j, :],
                        start=(j == 0),
                        stop=(j == KSUB - 1),
                    )
                nc.scalar.activation(
                    qT[:, m, c0 : c0 + 512], ps[:], AF.Identity,
                    bias=bqk_s[:, m : m + 1],
                )
        pool_w.release()

        # ======== attention ========
        pool_cr = tc.alloc_tile_pool(name="cr", bufs=1, side="right")
        ctx = pool_cr.tile([P, KSUB, SQ], BF16)
        lsum = pool_cr.tile([H, SQ], F32)
        for h in range(H):
            off = 64 * (h % 2)
            sub = h // 2
            q_h = qT[off : off + 64, sub, :]
            k_h = kT[off : off + 64, sub, :]
            for g in range(2):
                cps = psum_ctx.tile([HD + 1, 512], F32, tag="ctx")
                n_kt = 8 * g + 8
                for kt in range(n_kt):
                    first = max(4 * g, kt // 2)
                    qoff = (first - 4 * g) * 128
                    width = 512 - qoff
                    sps = psum_mm.tile([P, 512], F32, tag="mm")
                    nc.tensor.matmul(
                        sps[:, :width],
                        k_h[:, 128 * kt : 128 * (kt + 1)],
                        q_h[:, 512 * g + qoff : 512 * (g + 1)],
                        start=True,
                        stop=True,
                    )
                    pT = ptpool.tile([P, 512], BF16, tag="pt")
                    nc.scalar.activation(
                        pT[:, :width], sps[:, :width], AF.Exp, scale=SCALE
                    )
                    if kt // 2 >= 4 * g:
                        nc.vector.tensor_mul(pT[:, 0:P], pT[:, 0:P], mask_s[:, kt, :])
                    nc.tensor.matmul(
                        cps[:, qoff:512],
                        vaug[:, kt, h, :],
                        pT[:, :width],
                        start=(kt == 0),
                        stop=(kt == n_kt - 1),
                    )
                # evacuate raw ctx (bf16) and the l row (f32)
                gsl = slice(512 * g, 512 * (g + 1))
                if off == 0:
                    nc.vector.tensor_copy(ctx[0:HD, sub, gsl], cps[0:HD, :])
                else:
                    st = stage.tile([HD, 512], BF16, tag="cstage")
                    nc.vector.tensor_copy(st[:], cps[0:HD, :])
                    nc.sync.dma_start(out=ctx[off : off + 64, sub, gsl], in_=st[:])
                lt = stage.tile([HD + 1, 512], F32, tag="lstage")
                nc.vector.tensor_copy(lt[HD : HD + 1, :], cps[HD : HD + 1, :])
                nc.sync.dma_start(out=lsum[h : h + 1, gsl], in_=lt[HD : HD + 1, :])

        # softmax denominators -> reciprocals (12 lanes, one op)
        rsum = pool_cr.tile([H, SQ], F32)
        nc.vector.reciprocal(rsum[:], lsum[:])

        # normalize in place: ctx *= 1/l (head pairs share partition tiles)
        for sub in range(KSUB):
            for g in range(2):
                gsl = slice(512 * g, 512 * (g + 1))
                rtmp = stage.tile([2, 512], F32, tag="rtmp")
                nc.sync.dma_start(
                    out=rtmp[:], in_=rsum[2 * sub : 2 * sub + 2, gsl]
                )
                rb = psum_bc.tile([P, 512], F32, tag="bc")
                nc.tensor.matmul(rb[:], sel2[:], rtmp[:], start=True, stop=True)
                nc.vector.tensor_mul(ctx[:, sub, gsl], ctx[:, sub, gsl], rb[:])
        ptpool.release()

        # ======== out-proj + residual -> h1 (f32) ========
        pool_wo = tc.alloc_tile_pool(name="wo", bufs=1, side="right")
        h1 = h1pool.tile([P, KSUB, SQ], F32)
        wo_s = pool_wo.tile([P, KSUB, D], BF16)
        nc.sync.dma_start(out=wo_s[:], in_=wod[:])
        for c0 in range(0, SQ, 512):
            xres = lnx.tile([P, KSUB, 512], F32, tag="ln_x")
            nc.sync.dma_start(out=xres[:], in_=xTq[:, :, c0 : c0 + 512])
            for m in range(KSUB):
                ps = psum_mm.tile([P, 512], F32, tag="mm")
                for j in range(KSUB):
                    nc.tensor.matmul(
                        ps[:],
                        wo_s[:, j, 128 * m : 128 * (m + 1)],
                        ctx[:, j, c0 : c0 + 512],
                        start=(j == 0),
                        stop=(j == KSUB - 1),
                    )
                nc.vector.scalar_tensor_tensor(
                    out=h1[:, m, c0 : c0 + 512], in0=ps[:],
                    scalar=bo_s[:, m : m + 1], in1=xres[:, m, :],
                    op0=ALU.add, op1=ALU.add,
                )
        pool_wo.release()
        pool_cr.release()
        pool_attn.release()

        # ======== LN2 + MLP ========
        pool_mlp = tc.alloc_tile_pool(name="mlp", bufs=1, side="right")
        xn2 = pool_mlp.tile([P, KSUB, SQ], BF16)
        for c0 in range(0, SQ, 512):
            xnch = xn2[:, :, c0 : c0 + 512]
            _ln_stats_apply(
                nc, lnp, lnrows, psum_stat, psum_bc, h1[:, :, c0 : c0 + 512], xnch,
                ones, ones_row, eps_row,
            )

        # fc1 + gelu -> guT (bf16)
        w1_s = pool_mlp.tile([P, KSUB, 4 * D], BF16, tag="mlp_w")
        nc.sync.dma_start(out=w1_s[:], in_=w1d[:])
        guT = pool_mlp.tile([P, 4 * KSUB, SQ], BF16)
        for m in range(4 * KSUB):
            for c0 in range(0, SQ, 512):
                ps = psum_mm.tile([P, 512], F32, tag="mm")
                for j in range(KSUB):
                    nc.tensor.matmul(
                        ps[:],
                        w1_s[:, j, 128 * m : 128 * (m + 1)],
                        xn2[:, j, c0 : c0 + 512],
                        start=(j == 0),
                        stop=(j == KSUB - 1),
                    )
                nc.scalar.activation(
                    guT[:, m, c0 : c0 + 512], ps[:], AF.Gelu,
                    bias=b1_s[:, m : m + 1],
                )

        # fc2 + residual -> yT
        w2_s = pool_mlp.tile([P, 4 * KSUB, D], BF16, tag="mlp_w")
        nc.sync.dma_start(out=w2_s[:], in_=w2d[:])
        for m in range(KSUB):
            for c0 in range(0, SQ, 512):
                ps = psum_mm.tile([P, 512], F32, tag="mm")
                for j in range(4 * KSUB):
                    nc.tensor.matmul(
                        ps[:],
                        w2_s[:, j, 128 * m : 128 * (m + 1)],
                        guT[:, j, c0 : c0 + 512],
                        start=(j == 0),
                        stop=(j == 4 * KSUB - 1),
                    )
                yt = stage.tile([P, 512], F32, tag="ystage")
                nc.vector.scalar_tensor_tensor(
                    out=yt[:], in0=ps[:], scalar=b2_s[:, m : m + 1],
                    in1=h1[:, m, c0 : c0 + 512], op0=ALU.add, op1=ALU.add,
                )
                nc.sync.dma_start(out=yT[:, m, c0 : c0 + 512], in_=yt[:])

        pool_mlp.release()
        h1pool.release()
        stage.release()
        xnp.release()
        lnrows.release()
        lnx.release()
        lnp.release()
        persist.release()
        psum_bc.release()
        psum_stat.release()
        psum_ctx.release()
        psum_mm.release()
    return nc


_NC = None


def _get_nc():
    global _NC
    if _NC is None:
        _NC = build_nc()
    return _NC


def _feature_major(a2d):
    """[T, D'] -> [128, D'//128, T] with feature d at (d%128, d//128)."""
    t, d = a2d.shape
    return np.ascontiguousarray(a2d.T.reshape(d // P, P, t).transpose(1, 0, 2))


def _col_pack(vec):
    """[D'] -> [128, D'//128] with element d at (d%128, d//128)."""
    return np.ascontiguousarray(vec.reshape(-1, P).T)


def _prep_inputs(inputs):
    x = np.asarray(inputs["x"], np.float32)
    ln1_g = np.asarray(inputs["ln1_g"], np.float32)
    ln1_b = np.asarray(inputs["ln1_b"], np.float32)
    W_qkv = np.asarray(inputs["W_qkv"], np.float32)
    b_qkv = np.asarray(inputs["b_qkv"], np.float32)
    W_o = np.asarray(inputs["W_o"], np.float32)
    b_o = np.asarray(inputs["b_o"], np.float32)
    ln2_g = np.asarray(inputs["ln2_g"], np.float32)
    ln2_b = np.asarray(inputs["ln2_b"], np.float32)
    W1 = np.asarray(inputs["W1"], np.float32)
    b1 = np.asarray(inputs["b1"], np.float32)
    W2 = np.asarray(inputs["W2"], np.float32)
    b2 = np.asarray(inputs["b2"], np.float32)

    bf = ml_dtypes.bfloat16
    wqkv_g = (ln1_g[:, None] * W_qkv).astype(bf)
    wqkv_p = np.ascontiguousarray(
        wqkv_g.reshape(KSUB, P, 3 * D).transpose(1, 0, 2)
    )
    bqkv_f = b_qkv + ln1_b @ W_qkv            # folded LN1 shift
    bqk_p = _col_pack(bqkv_f[: 2 * D].astype(np.float32))   # [128, 12]
    b_v = bqkv_f[2 * D :]
    bo_f = b_o + b_v @ W_o                    # V bias folded into out-proj
    bo_p = _col_pack(bo_f.astype(np.float32))
    wo_p = np.ascontiguousarray(
        W_o.astype(bf).reshape(KSUB, P, D).transpose(1, 0, 2)
    )
    w1_g = (ln2_g[:, None] * W1).astype(bf)
    w1_p = np.ascontiguousarray(w1_g.reshape(KSUB, P, 4 * D).transpose(1, 0, 2))
    b1_f = b1 + ln2_b @ W1
    b1_p = _col_pack(b1_f.astype(np.float32))
    w2_p = np.ascontiguousarray(
        W2.astype(bf).reshape(4 * KSUB, P, D).transpose(1, 0, 2)
    )
    b2_p = _col_pack(b2.astype(np.float32))

    sel2_np = np.zeros((2, P), np.float32)
    sel2_np[0, :HD] = 1.0
    sel2_np[1, HD:] = 1.0
    in_maps = []
    for c in range(N_CORES):
        b, h = divmod(c, 2)
        blocks = own_blocks(h)
        tok = np.concatenate([np.arange(P * g, P * (g + 1)) for g in blocks])
        mask = np.zeros((NB, P, P), np.float32)
        for kt in range(NB):
            gq = blocks[kt // 2]
            krange = P * kt + np.arange(P)
            qrange = P * gq + np.arange(P)
            mask[kt] = (krange[:, None] <= qrange[None, :]).astype(np.float32)
        in_maps.append(
            {
                "xT": _feature_major(x[b]),
                "xTq": _feature_major(x[b][tok]),
                "wqkv": wqkv_p,
                "wo": wo_p,
                "w1": w1_p,
                "w2": w2_p,
                "bqk": bqk_p,
                "bo": bo_p,
                "b1": b1_p,
                "b2": b2_p,
                "mask": mask.astype(bf),
                "sel2": sel2_np,
            }
        )
    return in_maps


def _assemble(results):
    y = np.empty((B, S, D), np.float32)
    for c in range(N_CORES):
        b, h = divmod(c, 2)
        blocks = own_blocks(h)
        yt = results[c]["yT"]  # [128, 6, 1024]
        for j, g in enumerate(blocks):
            chunk = yt[:, :, P * j : P * (j + 1)]          # [128, 6, 128]
            y[b, P * g : P * (g + 1), :] = (
                chunk.transpose(1, 0, 2).reshape(D, P).T
            )
    return y


def _run(inputs, trace=False):
    nc = _get_nc()
    in_maps = _prep_inputs(inputs)
    res = run_bass_kernel_spmd(nc, in_maps, list(range(N_CORES)), trace=trace)
    return _assemble(res.results), res


def kernel(**inputs):
    out, _ = _run(inputs)
    return out
